# revision 4
# baseline (speedup 1.0000x reference)
"""Trainium2 Bass kernel for BiGNNLayer (COO SpMM + dense mix).

Computes, for L given in COO form (lap_rows=dest, lap_cols=src, lap_vals):
    x   = segment_sum(lap_vals * features[lap_cols], lap_rows)   # L @ F
    out = (features + x) @ W1 + b1 + (x * features) @ W2 + b2

Sharding: dest-node rows are split across 8 cores (12500 rows each); edges
are partitioned by dest core on the host; the feature table is replicated
into every core's HBM so no device collectives are needed.

Per-core SPMD kernel: dests are degree-sorted into 128-row tiles; each
tile's edges form a ragged slot matrix [128 x K_t] (K_t = max degree in
tile, shared across cores so the program is SPMD-uniform).  Slot column k
of tile t is fetched with one vector-offset DMA (128 dynamic row offsets,
one per partition - the only indirect-DMA form this hardware executes).
Slots are scaled by vals and reduced over k with a strided tensor_reduce.
The dense part transposes x tiles on the PE and runs two accumulating
matmuls with W1/W2 stationary, producing a transposed output tile that the
host un-permutes and un-transposes.
"""

import sys

sys.path.insert(0, "/opt/trn_rl_repo")

import numpy as np

import concourse.bacc as bacc
import concourse.tile as tile
from concourse import bass, mybir
from concourse.bass import IndirectOffsetOnAxis
from concourse.bass_utils import run_bass_kernel_spmd

# ---------------- problem constants (hardcoded per the contract) -----------
N_NODES = 100000
N_EDGES = 3200000
D = 64
CORES = 8
ND = N_NODES // CORES          # 12500 dest rows per core
T_ROWS = (ND + 127) // 128     # 98 row tiles (12544 padded rows)
NDP = T_ROWS * 128

FP32 = mybir.dt.float32
INT32 = mybir.dt.int32


# ---------------------------- host prep ------------------------------------
def _prep(lap_rows, lap_cols, lap_vals, features, W1, b1, W2, b2):
    lap_rows = np.ascontiguousarray(lap_rows)
    lap_cols = np.ascontiguousarray(lap_cols)
    lap_vals = np.ascontiguousarray(lap_vals)
    features = np.ascontiguousarray(features, dtype=np.float32)

    core = lap_rows // ND
    order = np.argsort(core, kind="stable")
    rows_s = lap_rows[order]
    cols_s = lap_cols[order]
    vals_s = lap_vals[order]
    bounds = np.searchsorted(core[order], np.arange(CORES + 1))

    # per-core degree-sorted permutation + per-tile max degree
    perms = []
    edge_data = []
    ktile = np.zeros((CORES, T_ROWS), np.int64)
    for c in range(CORES):
        lo, hi = bounds[c], bounds[c + 1]
        dloc = rows_s[lo:hi] - c * ND
        deg = np.bincount(dloc, minlength=ND)
        perm = np.argsort(-deg, kind="stable")          # descending degree
        degs = deg[perm]
        pad = np.zeros(NDP - ND, np.int64)
        degp = np.concatenate([degs, pad])
        ktile[c] = degp.reshape(T_ROWS, 128).max(axis=1)
        perms.append(perm)
        edge_data.append((dloc, cols_s[lo:hi], vals_s[lo:hi], deg, perm))

    kt = ktile.max(axis=0)                              # uniform across cores
    kt = np.maximum(kt, 1)                              # at least one slot
    offs = np.zeros(T_ROWS + 1, np.int64)
    np.cumsum(kt, out=offs[1:])
    ksum = int(offs[-1])

    bias = (np.asarray(b1, np.float32) + np.asarray(b2, np.float32)).reshape(D, 1)
    W1 = np.ascontiguousarray(W1, np.float32)
    W2 = np.ascontiguousarray(W2, np.float32)
    ident = np.eye(128, dtype=np.float32)

    in_maps = []
    for c in range(CORES):
        dloc, cols, vals, deg, perm = edge_data[c]
        # position of each dest in the degree-sorted order
        pos_of_dest = np.empty(ND, np.int64)
        pos_of_dest[perm] = np.arange(ND)
        # rank of each edge within its dest
        o2 = np.argsort(dloc, kind="stable")
        d2 = dloc[o2]
        starts = np.zeros(ND, np.int64)
        np.cumsum(deg[:-1], out=starts[1:])
        rank = np.arange(d2.shape[0]) - starts[d2]
        pos = pos_of_dest[d2]                            # sorted position
        t = pos // 128
        p = pos % 128
        gidx = np.zeros((128, ksum), np.int32)
        gval = np.zeros((128, ksum), np.float32)
        col_idx = offs[t] + rank
        gidx[p, col_idx] = cols[o2]
        gval[p, col_idx] = vals[o2]

        fT = np.zeros((D, NDP), np.float32)
        fT[:, :ND] = features[c * ND + perm].T

        in_maps.append(
            {
                "feat": features,
                "gidx": gidx,
                "gval": gval,
                "fT": fT,
                "W1": W1,
                "W2": W2,
                "bias": bias,
                "ident": ident,
            }
        )
    return in_maps, [e[4] for e in edge_data], kt.tolist(), offs.tolist(), ksum


# --------------------------- device kernel ---------------------------------
def build_kernel(kt, offs, ksum):
    nc = bacc.Bacc("TRN2", target_bir_lowering=False, debug=False)

    feat = nc.dram_tensor("feat", [N_NODES, D], FP32, kind="ExternalInput")
    gidx = nc.dram_tensor("gidx", [128, ksum], INT32, kind="ExternalInput")
    gval = nc.dram_tensor("gval", [128, ksum], FP32, kind="ExternalInput")
    fT = nc.dram_tensor("fT", [D, NDP], FP32, kind="ExternalInput")
    W1 = nc.dram_tensor("W1", [D, D], FP32, kind="ExternalInput")
    W2 = nc.dram_tensor("W2", [D, D], FP32, kind="ExternalInput")
    bias = nc.dram_tensor("bias", [D, 1], FP32, kind="ExternalInput")
    ident = nc.dram_tensor("ident", [128, 128], FP32, kind="ExternalInput")

    outT = nc.dram_tensor("outT", [D, NDP], FP32, kind="ExternalOutput")

    kmax = max(kt)

    with tile.TileContext(nc) as tc:
        with tc.tile_pool(name="acc", bufs=1) as apool:
            x_acc = apool.tile([128, T_ROWS * D], FP32)

            # ------------- phase A: ragged gather + scale + reduce ---------
            with (
                tc.tile_pool(name="gbuf", bufs=2) as gpool,
                tc.tile_pool(name="meta", bufs=1) as mpool,
            ):
                idx_sb = mpool.tile([128, ksum], INT32)
                nc.sync.dma_start(out=idx_sb[:], in_=gidx[:])
                val_sb = mpool.tile([128, ksum], FP32)
                nc.sync.dma_start(out=val_sb[:], in_=gval[:])

                for t in range(T_ROWS):
                    K = kt[t]
                    off = offs[t]
                    G = gpool.tile([128, kmax * D], FP32, tag="G")
                    for k in range(K):
                        nc.gpsimd.indirect_dma_start(
                            out=G[:, k * D : (k + 1) * D],
                            out_offset=None,
                            in_=feat[:],
                            in_offset=IndirectOffsetOnAxis(
                                ap=idx_sb[:, off + k : off + k + 1], axis=0
                            ),
                        )
                    G3 = G[:].rearrange("p (k f) -> p k f", f=D)[:, :K, :]
                    nc.vector.tensor_tensor(
                        out=G3,
                        in0=G3,
                        in1=val_sb[:, off : off + K, None].to_broadcast([128, K, D]),
                        op=mybir.AluOpType.mult,
                    )
                    gview = G[:].rearrange("p (k f) -> p f k", k=kmax, f=D)[:, :, :K]
                    nc.vector.tensor_reduce(
                        out=x_acc[:, t * D : (t + 1) * D],
                        in_=gview,
                        axis=mybir.AxisListType.X,
                        op=mybir.AluOpType.add,
                    )

        # ------------------ phase B: dense W1/W2 part ----------------------
            with (
                tc.tile_pool(name="dense", bufs=1) as dpool,
                tc.tile_pool(name="dwork", bufs=3) as wpool,
                tc.tile_pool(name="psum", bufs=4, space="PSUM") as pspool,
            ):
                fT_sb = dpool.tile([D, NDP], FP32)
                nc.sync.dma_start(out=fT_sb[:], in_=fT[:])
                w1_sb = dpool.tile([D, D], FP32)
                nc.sync.dma_start(out=w1_sb[:], in_=W1[:])
                w2_sb = dpool.tile([D, D], FP32)
                nc.sync.dma_start(out=w2_sb[:], in_=W2[:])
                bias_sb = dpool.tile([D, 1], FP32)
                nc.sync.dma_start(out=bias_sb[:], in_=bias[:])
                id_sb = dpool.tile([128, 128], FP32)
                nc.sync.dma_start(out=id_sb[:], in_=ident[:])
                outT_sb = dpool.tile([D, NDP], FP32)

                for i in range(T_ROWS):
                    xT_ps = pspool.tile([D, 128], FP32, tag="xT")
                    nc.tensor.transpose(
                        out=xT_ps[:],
                        in_=x_acc[:, i * D : (i + 1) * D],
                        identity=id_sb[:],
                    )
                    fslice = fT_sb[:, i * 128 : (i + 1) * 128]
                    a_t = wpool.tile([D, 128], FP32, tag="A")
                    nc.vector.tensor_tensor(
                        out=a_t[:], in0=fslice, in1=xT_ps[:], op=mybir.AluOpType.add
                    )
                    b_t = wpool.tile([D, 128], FP32, tag="B")
                    nc.vector.tensor_tensor(
                        out=b_t[:], in0=fslice, in1=xT_ps[:], op=mybir.AluOpType.mult
                    )
                    o_ps = pspool.tile([D, 128], FP32, tag="o")
                    nc.tensor.matmul(
                        o_ps[:], lhsT=w1_sb[:], rhs=a_t[:], start=True, stop=False
                    )
                    nc.tensor.matmul(
                        o_ps[:], lhsT=w2_sb[:], rhs=b_t[:], start=False, stop=True
                    )
                    nc.vector.tensor_scalar_add(
                        outT_sb[:, i * 128 : (i + 1) * 128],
                        o_ps[:],
                        bias_sb[:],
                    )
                nc.sync.dma_start(out=outT[:], in_=outT_sb[:])

    nc.compile()
    return nc


# ------------------------------ entry point --------------------------------
def kernel(lap_rows, lap_cols, lap_vals, features, W1, b1, W2, b2):
    in_maps, perms, kt, offs, ksum = _prep(
        lap_rows, lap_cols, lap_vals, features, W1, b1, W2, b2
    )
    nc = build_kernel(kt, offs, ksum)
    res = run_bass_kernel_spmd(nc, in_maps, core_ids=list(range(CORES)))
    out = np.empty((N_NODES, D), np.float32)
    for c in range(CORES):
        got = res.results[c]["outT"][:, :ND].T      # [ND, D], degree-sorted order
        out[c * ND + perms[c]] = got
    return out


if __name__ == "__main__":
    import reference

    inp = reference.setup_inputs()
    inp = {k: np.asarray(v) for k, v in inp.items()}
    got = kernel(**inp)
    print("kernel ran, output shape", got.shape)


# revision 5
# speedup vs baseline: 1.0285x; 1.0285x over previous
"""Trainium2 Bass kernel for BiGNNLayer (COO SpMM + dense mix).

Computes, for L given in COO form (lap_rows=dest, lap_cols=src, lap_vals):
    x   = segment_sum(lap_vals * features[lap_cols], lap_rows)   # L @ F
    out = (features + x) @ W1 + b1 + (x * features) @ W2 + b2

Sharding: dest nodes are striped across the 8 cores by global degree rank
(rank r -> core r%8, position r//8), so every core gets exactly 12500 dests
with near-identical degree profiles; edges are partitioned by dest core on
the host; the feature table is replicated into every core's HBM so no
device collectives are needed.

Per-core SPMD kernel: positions are degree-sorted 128-row tiles; each
tile's edges form a ragged slot matrix [128 x K_t] where K_t is the global
rank-1024t degree (identical across cores, keeping the program SPMD-
uniform and the total slot-column count within ~0.2% of the edges/128
floor).  Slot column k of tile t is fetched with one vector-offset DMA
(128 dynamic int32 row offsets, one per partition - the only indirect-DMA
form this hardware executes).  Slots are scaled by vals and reduced over k
with a strided tensor_reduce.  The dense part (PE transpose of x tiles +
two accumulating matmuls with W1/W2 stationary) is emitted with live
buffers alongside phase A so the scheduler hides it under the gather
stream; the host un-permutes and un-transposes the output.
"""

import sys

sys.path.insert(0, "/opt/trn_rl_repo")

import numpy as np

import concourse.bacc as bacc
import concourse.tile as tile
from concourse import bass, mybir
from concourse.bass import IndirectOffsetOnAxis
from concourse.bass_utils import run_bass_kernel_spmd

# ---------------- problem constants (hardcoded per the contract) -----------
N_NODES = 100000
N_EDGES = 3200000
D = 64
CORES = 8
ND = N_NODES // CORES          # 12500 dest rows per core
T_ROWS = (ND + 127) // 128     # 98 row tiles (12544 padded rows)
NDP = T_ROWS * 128

FP32 = mybir.dt.float32
INT32 = mybir.dt.int32


# ---------------------------- host prep ------------------------------------
def _prep(lap_rows, lap_cols, lap_vals, features, W1, b1, W2, b2):
    lap_rows = np.ascontiguousarray(lap_rows)
    lap_cols = np.ascontiguousarray(lap_cols)
    lap_vals = np.ascontiguousarray(lap_vals)
    features = np.ascontiguousarray(features, dtype=np.float32)

    # global degree-rank striping: rank r -> core r%8, position r//8
    deg = np.bincount(lap_rows, minlength=N_NODES)
    gorder = np.argsort(-deg, kind="stable")
    grank = np.empty(N_NODES, np.int64)
    grank[gorder] = np.arange(N_NODES)
    core_of = (grank % CORES).astype(np.int64)
    pos_of = grank // CORES                      # 0..ND-1 within core

    # K_t identical across cores: tile t's max degree = degree at rank 1024t
    deg_sorted = deg[gorder]
    kt = np.maximum(deg_sorted[np.arange(T_ROWS) * 128 * CORES], 1)
    offs = np.zeros(T_ROWS + 1, np.int64)
    np.cumsum(kt, out=offs[1:])
    ksum = int(offs[-1])

    ecore = core_of[lap_rows]
    order = np.argsort(ecore, kind="stable")
    bounds = np.searchsorted(ecore[order], np.arange(CORES + 1))
    pos_s = pos_of[lap_rows[order]]
    cols_s = lap_cols[order]
    vals_s = lap_vals[order]

    bias = (np.asarray(b1, np.float32) + np.asarray(b2, np.float32)).reshape(D, 1)
    W1 = np.ascontiguousarray(W1, np.float32)
    W2 = np.ascontiguousarray(W2, np.float32)
    ident = np.eye(128, dtype=np.float32)

    in_maps = []
    perms = []
    for c in range(CORES):
        lo, hi = bounds[c], bounds[c + 1]
        pos = pos_s[lo:hi]                       # dest position within core
        cols = cols_s[lo:hi]
        vals = vals_s[lo:hi]
        o2 = np.argsort(pos, kind="stable")
        p2 = pos[o2]
        cdeg = np.bincount(p2, minlength=ND)
        starts = np.zeros(ND, np.int64)
        np.cumsum(cdeg[:-1], out=starts[1:])
        rank = np.arange(p2.shape[0]) - starts[p2]

        t = p2 // 128
        p = p2 % 128
        if np.any(rank >= kt[t]):
            raise AssertionError("slot overflow: degree exceeds tile budget")
        gidx = np.zeros((128, ksum), np.int32)
        gval = np.zeros((128, ksum), np.float32)
        col_idx = offs[t] + rank
        gidx[p, col_idx] = cols[o2]
        gval[p, col_idx] = vals[o2]

        perm = gorder[c::CORES]                  # position i -> dest id
        fT = np.zeros((D, NDP), np.float32)
        fT[:, :ND] = features[perm].T
        perms.append(perm)

        in_maps.append(
            {
                "feat": features,
                "gidx": gidx,
                "gval": gval,
                "fT": fT,
                "W1": W1,
                "W2": W2,
                "bias": bias,
                "ident": ident,
            }
        )
    return in_maps, perms, kt.tolist(), offs.tolist(), ksum


# --------------------------- device kernel ---------------------------------
def build_kernel(kt, offs, ksum):
    nc = bacc.Bacc("TRN2", target_bir_lowering=False, debug=False)

    feat = nc.dram_tensor("feat", [N_NODES, D], FP32, kind="ExternalInput")
    gidx = nc.dram_tensor("gidx", [128, ksum], INT32, kind="ExternalInput")
    gval = nc.dram_tensor("gval", [128, ksum], FP32, kind="ExternalInput")
    fT = nc.dram_tensor("fT", [D, NDP], FP32, kind="ExternalInput")
    W1 = nc.dram_tensor("W1", [D, D], FP32, kind="ExternalInput")
    W2 = nc.dram_tensor("W2", [D, D], FP32, kind="ExternalInput")
    bias = nc.dram_tensor("bias", [D, 1], FP32, kind="ExternalInput")
    ident = nc.dram_tensor("ident", [128, 128], FP32, kind="ExternalInput")

    outT = nc.dram_tensor("outT", [D, NDP], FP32, kind="ExternalOutput")

    kmax = max(kt)

    with tile.TileContext(nc) as tc:
        with (
            tc.tile_pool(name="acc", bufs=1) as apool,
            tc.tile_pool(name="dense", bufs=1) as dpool,
            tc.tile_pool(name="gbuf", bufs=2) as gpool,
            tc.tile_pool(name="meta", bufs=1) as mpool,
            tc.tile_pool(name="dwork", bufs=3) as wpool,
            tc.tile_pool(name="psum", bufs=4, space="PSUM") as pspool,
        ):
            x_acc = apool.tile([128, T_ROWS * D], FP32)

            idx_sb = mpool.tile([128, ksum], INT32)
            nc.sync.dma_start(out=idx_sb[:], in_=gidx[:])
            val_sb = mpool.tile([128, ksum], FP32)
            nc.sync.dma_start(out=val_sb[:], in_=gval[:])

            fT_sb = dpool.tile([D, NDP], FP32)
            nc.sync.dma_start(out=fT_sb[:], in_=fT[:])
            w1_sb = dpool.tile([D, D], FP32)
            nc.sync.dma_start(out=w1_sb[:], in_=W1[:])
            w2_sb = dpool.tile([D, D], FP32)
            nc.sync.dma_start(out=w2_sb[:], in_=W2[:])
            bias_sb = dpool.tile([D, 1], FP32)
            nc.sync.dma_start(out=bias_sb[:], in_=bias[:])
            id_sb = dpool.tile([128, 128], FP32)
            nc.sync.dma_start(out=id_sb[:], in_=ident[:])
            outT_sb = dpool.tile([D, NDP], FP32)

            def dense_tile(i):
                xT_ps = pspool.tile([D, 128], FP32, tag="xT")
                nc.tensor.transpose(
                    out=xT_ps[:],
                    in_=x_acc[:, i * D : (i + 1) * D],
                    identity=id_sb[:],
                )
                fslice = fT_sb[:, i * 128 : (i + 1) * 128]
                a_t = wpool.tile([D, 128], FP32, tag="A")
                nc.vector.tensor_tensor(
                    out=a_t[:], in0=fslice, in1=xT_ps[:], op=mybir.AluOpType.add
                )
                b_t = wpool.tile([D, 128], FP32, tag="B")
                nc.vector.tensor_tensor(
                    out=b_t[:], in0=fslice, in1=xT_ps[:], op=mybir.AluOpType.mult
                )
                o_ps = pspool.tile([D, 128], FP32, tag="o")
                nc.tensor.matmul(
                    o_ps[:], lhsT=w1_sb[:], rhs=a_t[:], start=True, stop=False
                )
                nc.tensor.matmul(
                    o_ps[:], lhsT=w2_sb[:], rhs=b_t[:], start=False, stop=True
                )
                nc.vector.tensor_scalar_add(
                    outT_sb[:, i * 128 : (i + 1) * 128],
                    o_ps[:],
                    bias_sb[:],
                )

            for t in range(T_ROWS):
                K = kt[t]
                off = offs[t]
                G = gpool.tile([128, kmax * D], FP32, tag="G")
                for k in range(K):
                    nc.gpsimd.indirect_dma_start(
                        out=G[:, k * D : (k + 1) * D],
                        out_offset=None,
                        in_=feat[:],
                        in_offset=IndirectOffsetOnAxis(
                            ap=idx_sb[:, off + k : off + k + 1], axis=0
                        ),
                    )
                G3 = G[:].rearrange("p (k f) -> p k f", f=D)[:, :K, :]
                nc.vector.tensor_tensor(
                    out=G3,
                    in0=G3,
                    in1=val_sb[:, off : off + K, None].to_broadcast([128, K, D]),
                    op=mybir.AluOpType.mult,
                )
                gview = G[:].rearrange("p (k f) -> p f k", k=kmax, f=D)[:, :, :K]
                nc.vector.tensor_reduce(
                    out=x_acc[:, t * D : (t + 1) * D],
                    in_=gview,
                    axis=mybir.AxisListType.X,
                    op=mybir.AluOpType.add,
                )
                dense_tile(t)

            nc.sync.dma_start(out=outT[:], in_=outT_sb[:])

    nc.compile()
    return nc


# ------------------------------ entry point --------------------------------
def kernel(lap_rows, lap_cols, lap_vals, features, W1, b1, W2, b2):
    in_maps, perms, kt, offs, ksum = _prep(
        lap_rows, lap_cols, lap_vals, features, W1, b1, W2, b2
    )
    nc = build_kernel(kt, offs, ksum)
    res = run_bass_kernel_spmd(nc, in_maps, core_ids=list(range(CORES)))
    out = np.empty((N_NODES, D), np.float32)
    for c in range(CORES):
        out[perms[c]] = res.results[c]["outT"][:, :ND].T
    return out


if __name__ == "__main__":
    import reference

    inp = reference.setup_inputs()
    inp = {k: np.asarray(v) for k, v in inp.items()}
    got = kernel(**inp)
    print("kernel ran, output shape", got.shape)


# revision 8
# speedup vs baseline: 1.0311x; 1.0025x over previous
"""Trainium2 Bass kernel for BiGNNLayer (COO SpMM + dense mix).

Computes, for L given in COO form (lap_rows=dest, lap_cols=src, lap_vals):
    x   = segment_sum(lap_vals * features[lap_cols], lap_rows)   # L @ F
    out = (features + x) @ W1 + b1 + (x * features) @ W2 + b2

Sharding: dest nodes are striped across the 8 cores by global degree rank
(rank r -> core r%8, position r//8), so every core gets exactly 12500 dests
with near-identical degree profiles; edges are partitioned by dest core on
the host; the feature table is replicated into every core's HBM so no
device collectives are needed.

Per-core SPMD kernel: positions are degree-sorted 128-row tiles; each
tile's edges form a ragged slot matrix [128 x K_t] where K_t is the global
rank-1024t degree (identical across cores, keeping the program SPMD-
uniform and the total slot-column count within ~0.2% of the edges/128
floor).  Slot column k of tile t is fetched with one vector-offset DMA
(128 dynamic int32 row offsets, one per partition - the only indirect-DMA
form this hardware executes).  Slots are scaled by vals and reduced over k
with a strided tensor_reduce.  The dense part (PE transpose of x tiles +
two accumulating matmuls with W1/W2 stationary) is emitted with live
buffers alongside phase A so the scheduler hides it under the gather
stream; the host un-permutes and un-transposes the output.
"""

import sys

sys.path.insert(0, "/opt/trn_rl_repo")

import numpy as np

import concourse.bacc as bacc
import concourse.tile as tile
from concourse import bass, mybir
from concourse.bass import IndirectOffsetOnAxis
from concourse.bass_utils import run_bass_kernel_spmd

# ---------------- problem constants (hardcoded per the contract) -----------
N_NODES = 100000
N_EDGES = 3200000
D = 64
CORES = 8
ND = N_NODES // CORES          # 12500 dest rows per core
T_ROWS = (ND + 127) // 128     # 98 row tiles (12544 padded rows)
NDP = T_ROWS * 128

FP32 = mybir.dt.float32
INT32 = mybir.dt.int32


# ---------------------------- host prep ------------------------------------
def _prep(lap_rows, lap_cols, lap_vals, features, W1, b1, W2, b2):
    lap_rows = np.ascontiguousarray(lap_rows)
    lap_cols = np.ascontiguousarray(lap_cols)
    lap_vals = np.ascontiguousarray(lap_vals)
    features = np.ascontiguousarray(features, dtype=np.float32)

    # global degree-rank striping: rank r -> core r%8, position r//8
    deg = np.bincount(lap_rows, minlength=N_NODES)
    gorder = np.argsort(-deg, kind="stable")
    grank = np.empty(N_NODES, np.int64)
    grank[gorder] = np.arange(N_NODES)
    core_of = (grank % CORES).astype(np.int64)
    pos_of = grank // CORES                      # 0..ND-1 within core

    # K_t identical across cores: tile t's max degree = degree at rank 1024t
    deg_sorted = deg[gorder]
    kt = np.maximum(deg_sorted[np.arange(T_ROWS) * 128 * CORES], 1)
    offs = np.zeros(T_ROWS + 1, np.int64)
    np.cumsum(kt, out=offs[1:])
    ksum = int(offs[-1])

    ecore = core_of[lap_rows]
    order = np.argsort(ecore, kind="stable")
    bounds = np.searchsorted(ecore[order], np.arange(CORES + 1))
    pos_s = pos_of[lap_rows[order]]
    cols_s = lap_cols[order]
    vals_s = lap_vals[order]

    bias = (np.asarray(b1, np.float32) + np.asarray(b2, np.float32)).reshape(D, 1)
    W1 = np.ascontiguousarray(W1, np.float32)
    W2 = np.ascontiguousarray(W2, np.float32)
    ident = np.eye(128, dtype=np.float32)

    in_maps = []
    perms = []
    for c in range(CORES):
        lo, hi = bounds[c], bounds[c + 1]
        pos = pos_s[lo:hi]                       # dest position within core
        cols = cols_s[lo:hi]
        vals = vals_s[lo:hi]
        o2 = np.argsort(pos, kind="stable")
        p2 = pos[o2]
        cdeg = np.bincount(p2, minlength=ND)
        starts = np.zeros(ND, np.int64)
        np.cumsum(cdeg[:-1], out=starts[1:])
        rank = np.arange(p2.shape[0]) - starts[p2]

        t = p2 // 128
        p = p2 % 128
        if np.any(rank >= kt[t]):
            raise AssertionError("slot overflow: degree exceeds tile budget")
        gidx = np.zeros((128, ksum), np.int32)
        gval = np.zeros((128, ksum), np.float32)
        col_idx = offs[t] + rank
        gidx[p, col_idx] = cols[o2]
        gval[p, col_idx] = vals[o2]

        perm = gorder[c::CORES]                  # position i -> dest id
        fT = np.zeros((D, NDP), np.float32)
        fT[:, :ND] = features[perm].T
        perms.append(perm)

        in_maps.append(
            {
                "feat": features,
                "gidx": gidx,
                "gval": gval,
                "fT": fT,
                "W1": W1,
                "W2": W2,
                "bias": bias,
                "ident": ident,
            }
        )
    return in_maps, perms, kt.tolist(), offs.tolist(), ksum


# --------------------------- device kernel ---------------------------------
def build_kernel(kt, offs, ksum):
    nc = bacc.Bacc("TRN2", target_bir_lowering=False, debug=False)

    feat = nc.dram_tensor("feat", [N_NODES, D], FP32, kind="ExternalInput")
    gidx = nc.dram_tensor("gidx", [128, ksum], INT32, kind="ExternalInput")
    gval = nc.dram_tensor("gval", [128, ksum], FP32, kind="ExternalInput")
    fT = nc.dram_tensor("fT", [D, NDP], FP32, kind="ExternalInput")
    W1 = nc.dram_tensor("W1", [D, D], FP32, kind="ExternalInput")
    W2 = nc.dram_tensor("W2", [D, D], FP32, kind="ExternalInput")
    bias = nc.dram_tensor("bias", [D, 1], FP32, kind="ExternalInput")
    ident = nc.dram_tensor("ident", [128, 128], FP32, kind="ExternalInput")

    outT = nc.dram_tensor("outT", [D, NDP], FP32, kind="ExternalOutput")

    kmax = max(kt)

    with tile.TileContext(nc) as tc:
        with (
            tc.tile_pool(name="acc", bufs=1) as apool,
            tc.tile_pool(name="dense", bufs=1) as dpool,
            tc.tile_pool(name="gbuf", bufs=2) as gpool,
            tc.tile_pool(name="meta", bufs=1) as mpool,
            tc.tile_pool(name="dwork", bufs=3) as wpool,
            tc.tile_pool(name="psum", bufs=4, space="PSUM") as pspool,
        ):
            x_acc = apool.tile([128, T_ROWS * D], FP32)

            # split the offset load so tile 0's gathers start immediately
            k0 = kt[0]
            idx_sb = mpool.tile([128, ksum], INT32)
            nc.sync.dma_start(out=idx_sb[:, :k0], in_=gidx[:, :k0])
            nc.sync.dma_start(out=idx_sb[:, k0:], in_=gidx[:, k0:])
            val_sb = mpool.tile([128, ksum], FP32)
            nc.sync.dma_start(out=val_sb[:], in_=gval[:])

            fT_sb = dpool.tile([D, NDP], FP32)
            nc.sync.dma_start(out=fT_sb[:], in_=fT[:])
            w1_sb = dpool.tile([D, D], FP32)
            nc.sync.dma_start(out=w1_sb[:], in_=W1[:])
            w2_sb = dpool.tile([D, D], FP32)
            nc.sync.dma_start(out=w2_sb[:], in_=W2[:])
            bias_sb = dpool.tile([D, 1], FP32)
            nc.sync.dma_start(out=bias_sb[:], in_=bias[:])
            id_sb = dpool.tile([128, 128], FP32)
            nc.sync.dma_start(out=id_sb[:], in_=ident[:])
            outT_sb = dpool.tile([D, NDP], FP32)

            def dense_tile(i):
                xT_ps = pspool.tile([D, 128], FP32, tag="xT")
                nc.tensor.transpose(
                    out=xT_ps[:],
                    in_=x_acc[:, i * D : (i + 1) * D],
                    identity=id_sb[:],
                )
                fslice = fT_sb[:, i * 128 : (i + 1) * 128]
                a_t = wpool.tile([D, 128], FP32, tag="A")
                nc.vector.tensor_tensor(
                    out=a_t[:], in0=fslice, in1=xT_ps[:], op=mybir.AluOpType.add
                )
                b_t = wpool.tile([D, 128], FP32, tag="B")
                nc.vector.tensor_tensor(
                    out=b_t[:], in0=fslice, in1=xT_ps[:], op=mybir.AluOpType.mult
                )
                o_ps = pspool.tile([D, 128], FP32, tag="o")
                nc.tensor.matmul(
                    o_ps[:], lhsT=w1_sb[:], rhs=a_t[:], start=True, stop=False
                )
                nc.tensor.matmul(
                    o_ps[:], lhsT=w2_sb[:], rhs=b_t[:], start=False, stop=True
                )
                nc.vector.tensor_scalar_add(
                    outT_sb[:, i * 128 : (i + 1) * 128],
                    o_ps[:],
                    bias_sb[:],
                )
                # stream this tile's output out immediately (hides the
                # final store under the remaining gather stream)
                nc.sync.dma_start(
                    out=outT[:, i * 128 : (i + 1) * 128],
                    in_=outT_sb[:, i * 128 : (i + 1) * 128],
                )

            for t in range(T_ROWS):
                K = kt[t]
                off = offs[t]
                G = gpool.tile([128, kmax * D], FP32, tag="G")
                for k in range(K):
                    nc.gpsimd.indirect_dma_start(
                        out=G[:, k * D : (k + 1) * D],
                        out_offset=None,
                        in_=feat[:],
                        in_offset=IndirectOffsetOnAxis(
                            ap=idx_sb[:, off + k : off + k + 1], axis=0
                        ),
                    )
                G3 = G[:].rearrange("p (k f) -> p k f", f=D)[:, :K, :]
                nc.vector.tensor_tensor(
                    out=G3,
                    in0=G3,
                    in1=val_sb[:, off : off + K, None].to_broadcast([128, K, D]),
                    op=mybir.AluOpType.mult,
                )
                gview = G[:].rearrange("p (k f) -> p f k", k=kmax, f=D)[:, :, :K]
                nc.vector.tensor_reduce(
                    out=x_acc[:, t * D : (t + 1) * D],
                    in_=gview,
                    axis=mybir.AxisListType.X,
                    op=mybir.AluOpType.add,
                )
                dense_tile(t)

    nc.compile()
    return nc


# ------------------------------ entry point --------------------------------
def kernel(lap_rows, lap_cols, lap_vals, features, W1, b1, W2, b2):
    in_maps, perms, kt, offs, ksum = _prep(
        lap_rows, lap_cols, lap_vals, features, W1, b1, W2, b2
    )
    nc = build_kernel(kt, offs, ksum)
    res = run_bass_kernel_spmd(nc, in_maps, core_ids=list(range(CORES)))
    out = np.empty((N_NODES, D), np.float32)
    for c in range(CORES):
        out[perms[c]] = res.results[c]["outT"][:, :ND].T
    return out


if __name__ == "__main__":
    import reference

    inp = reference.setup_inputs()
    inp = {k: np.asarray(v) for k, v in inp.items()}
    got = kernel(**inp)
    print("kernel ran, output shape", got.shape)
